# revision 1
# baseline (speedup 1.0000x reference)
"""Channel-attention (nn_ChannelAttentionModule) Trainium2 kernel.

Math (per batch b):
    X = x[b]  [C, N]  with C=512, N=64*64=4096
    q = Wq X + bq ; k = Wk X + bk ; v = Wv X + bv
    L = q k^T                       [C, C]
    out = softmax(L, -1) v + X      [C, N]

Key restructure: L = Wq G Wk^T + bq (Wk S + N bk)^T + (Wq S) bk^T  (outer
products), where G = X X^T (Gram, symmetric) and S = X 1 (row sums).
G is computed in a single fp16 pass (~11-bit input mantissa, 1 cyc/row
on the PE, fp32 PSUM accumulation) over the upper block-triangle,
mirrored via PE transposes; the two 512^3 projection matmuls run in
true fp32; the v-path runs in fp16.  Softmax logits stay fp32.

Sharding: pure data-parallel, one batch per NeuronCore (B=8, 8 cores).
"""

import numpy as np

import concourse.mybir as mybir
import concourse.tile as tile
from concourse import bacc
from concourse.bass_utils import run_bass_kernel_spmd

F32 = mybir.dt.float32
F32R = mybir.dt.float32r
F16 = mybir.dt.float16
AX = mybir.AxisListType.X
EXP = mybir.ActivationFunctionType.Exp

B = 8
C = 512
HW = 64 * 64
P = 128
CH = C // P  # 4 channel chunks
NT = HW // 512  # 8 spatial tiles of 512
NG = 8  # xtr granules (4 spatial chunks each)
# upper-triangle start per G row chunk
USTART = [0, 128, 256, 256]


def _body(tc, nc, io):
    xt16, x16 = io["xt16"], io["x16"]
    wqh, wql, wkh, wkl, wvt = io["wqh"], io["wql"], io["wkh"], io["wkl"], io["wvt"]
    bqr, bkr, nbkr, bvc = io["bqr"], io["bkr"], io["nbkr"], io["bvc"]
    id16, out = io["id16"], io["out"]

    ps = tc.alloc_tile_pool(name="ps", bufs=1, space="PSUM")
    sb = tc.alloc_tile_pool(name="sb", bufs=1)
    st = tc.alloc_tile_pool(name="st", bufs=3)
    so = tc.alloc_tile_pool(name="so", bufs=2)

    wv_sb = sb.tile([P, CH * C], F16, name="wv_sb", tag="wv_sb")
    bv_sb = sb.tile([P, CH], F32, name="bv_sb", tag="bv_sb")
    x16_sb = [sb.tile([P, HW], F16, name=f"x16_{i}", tag=f"x16_{i}") for i in range(CH)]
    v_sb = [sb.tile([P, HW], F16, name=f"vsb{i}", tag=f"vsb{i}") for i in range(CH)]
    wqh_sb = sb.tile([P, CH * C], F16, name="wqh_sb", tag="wqh_sb")
    wql_sb = sb.tile([P, CH * C], F16, name="wql_sb", tag="wql_sb")
    wkh_sb = sb.tile([P, CH * C], F16, name="wkh_sb", tag="wkh_sb")
    wkl_sb = sb.tile([P, CH * C], F16, name="wkl_sb", tag="wkl_sb")

    def wslice(tile_, e, lo, hi):
        return tile_[:, e * C + lo : e * C + hi]

    def v_conv(nt, tag):
        for o in range(CH):
            v_ps = ps.tile([P, 512], F32, name=f"vps{o}", tag=f"{tag}{o}")
            for c in range(CH):
                nc.tensor.matmul(
                    v_ps,
                    lhsT=wslice(wv_sb, c, o * P, (o + 1) * P),
                    rhs=x16_sb[c][:, nt * 512 : (nt + 1) * 512],
                    start=c == 0,
                    stop=c == CH - 1,
                )
            nc.vector.tensor_scalar_add(
                v_sb[o][:, nt * 512 : (nt + 1) * 512], v_ps, bv_sb[:, o : o + 1]
            )

    # ---- interleaved front: x16/xtr stream + v-conv/G rounds ----
    ar_sb = [
        sb.tile([P, 4 * C], F16, name=f"ar{g}", tag=f"ar{g}") for g in range(NG)
    ]
    xtr3 = xt16.rearrange("(g t p) c -> g p t c", p=P, t=4)
    g_ps = [ps.tile([P, C], F32, name=f"gps{i}", tag=f"pa{i}") for i in range(CH)]

    def x16_load(nt2):
        for c in range(CH):
            nc.gpsimd.dma_start(
                x16_sb[c][:, nt2 * 1024 : (nt2 + 1) * 1024],
                x16[c * P : (c + 1) * P, nt2 * 1024 : (nt2 + 1) * 1024],
            )

    def xtr_load(g2):
        nc.sync.dma_start(ar_sb[g2].rearrange("p (t c) -> p t c", t=4), xtr3[g2])

    def g_pass(g2):
        ar4 = ar_sb[g2]
        for t in range(4):
            n = g2 * 4 + t
            first, last = n == 0, n == 4 * NG - 1
            for c in range(CH):
                u = USTART[c]
                nc.tensor.matmul(
                    g_ps[c][:, u:],
                    lhsT=ar4[:, t * C + c * P : t * C + (c + 1) * P],
                    rhs=ar4[:, t * C + u : (t + 1) * C],
                    start=first,
                    stop=last,
                )

    nc.sync.dma_start(
        ar_sb[0][:, 0 : 2 * C].rearrange("p (t c) -> p t c", t=2), xtr3[0][:, 0:2]
    )
    nc.sync.dma_start(
        ar_sb[0][:, 2 * C :].rearrange("p (t c) -> p t c", t=2), xtr3[0][:, 2:4]
    )
    xtr_load(1)
    nc.sync.dma_start(
        wv_sb.rearrange("p (e c) -> p e c", e=CH),
        wvt.rearrange("(e p) c -> p e c", p=P),
    )
    nc.sync.dma_start(
        bv_sb.rearrange("p (e o) -> p e o", e=CH),
        bvc.rearrange("(e p) o -> p e o", p=P),
    )
    x16_load(0)
    g_pass(0)
    g_pass(1)
    xtr_load(2)
    xtr_load(3)
    x16_load(1)
    v_conv(0, "pb")
    v_conv(1, "pb")
    g_pass(2)
    g_pass(3)
    xtr_load(4)
    xtr_load(5)
    x16_load(2)
    v_conv(2, "pb")
    v_conv(3, "pb")
    g_pass(4)
    g_pass(5)
    xtr_load(6)
    xtr_load(7)
    for c in range(CH):
        nc.sync.dma_start(
            x16_sb[c][:, 3 * 1024 : 4 * 1024],
            x16[c * P : (c + 1) * P, 3 * 1024 : 4 * 1024],
        )
    for wtile, wdram in ((wqh_sb, wqh), (wkh_sb, wkh), (wql_sb, wql), (wkl_sb, wkl)):
        nc.sync.dma_start(
            wtile.rearrange("p (e c) -> p e c", e=CH),
            wdram.rearrange("(e p) c -> p e c", p=P),
        )
    v_conv(4, "pb")
    v_conv(5, "pb")
    s_col = [sb.tile([P, 1], F32, name=f"s{i}", tag=f"s{i}") for i in range(CH)]
    for i in range(CH):
        nc.vector.reduce_sum(s_col[i], x16_sb[i], axis=AX)
    g_pass(6)
    g_pass(7)
    v_conv(6, "pb")

    # ---- consts needed by the mid/late phases ----
    id16_sb = sb.tile([P, P], F16, name="id16sb", tag="id16sb")
    nc.sync.dma_start(id16_sb, id16)
    nbkr_sb = sb.tile([1, C], F32, name="nbkrsb", tag="nbkrsb")
    nc.sync.dma_start(nbkr_sb, nbkr)

    # ---- u1 = (Wq S)^T, u2 = (Wk S)^T (fp16-hi; error ~1e-4 on logits) ----
    s16 = [sb.tile([P, 1], F16, name=f"s16_{i}", tag=f"s16_{i}") for i in range(CH)]
    for i in range(CH):
        nc.scalar.copy(s16[i], s_col[i])
    u1_ps = ps.tile([1, C], F32, name="u1ps", tag="pb0")
    u2_ps = ps.tile([1, C], F32, name="u2ps", tag="pb1")
    for e in range(CH):
        nc.tensor.matmul(
            u1_ps, lhsT=s16[e], rhs=wslice(wqh_sb, e, 0, C),
            start=e == 0, stop=e == CH - 1,
        )
    for e in range(CH):
        nc.tensor.matmul(
            u2_ps, lhsT=s16[e], rhs=wslice(wkh_sb, e, 0, C),
            start=e == 0, stop=e == CH - 1,
        )

    # ---- G split straight from PSUM: gh = f16(G), gl = G - gh; lower
    #      blocks mirrored with f16 PE transposes (exact: transpose of the
    #      rounded equals rounding of the transpose for symmetric G) ----
    gh = [sb.tile([P, C], F16, name=f"gh{i}", tag=f"gh{i}") for i in range(CH)]
    gl = [sb.tile([P, C], F16, name=f"gl{i}", tag=f"gl{i}") for i in range(CH)]
    for c in range(CH):
        u = USTART[c]
        nc.scalar.copy(gh[c][:, u:], g_ps[c][:, u:])
        nc.vector.tensor_sub(gl[c][:, u:], g_ps[c][:, u:], gh[c][:, u:])
        for d in range(u // P):
            tbh = ps.tile([P, P], F16, name="tbh", tag=f"pb{2 + (c + d) % 2}")
            nc.tensor.transpose(tbh, gh[d][:, c * P : (c + 1) * P], id16_sb)
            nc.scalar.copy(gh[c][:, d * P : (d + 1) * P], tbh)
            tbl = ps.tile([P, P], F16, name="tbl", tag=f"pb{2 + (c + d + 1) % 2}")
            nc.tensor.transpose(tbl, gl[d][:, c * P : (c + 1) * P], id16_sb)
            nc.vector.tensor_copy(gl[c][:, d * P : (d + 1) * P], tbl)

    u1_sb = sb.tile([1, C], F32, name="u1_sb", tag="u1_sb")
    nc.vector.tensor_copy(u1_sb, u1_ps)
    lhs2 = sb.tile([2, C], F32, name="lhs2", tag="lhs2")
    nc.sync.dma_start(lhs2[0:1, :], bqr)
    nc.sync.dma_start(lhs2[1:2, :], u1_sb)
    rhs2 = sb.tile([2, C], F32, name="rhs2", tag="rhs2")
    nc.vector.tensor_add(rhs2[0:1, :], u2_ps, nbkr_sb)
    nc.sync.dma_start(rhs2[1:2, :], bkr)

    # ---- T1 = G Wk^T via 3 f16 passes (hi*hi + hi*lo + lo*hi), f-outer ----
    t1_ps = [ps.tile([P, C], F32, name=f"t1ps{i}", tag=f"pa{i}") for i in range(CH)]
    for f in range(CH):
        for e in range(CH):
            nc.tensor.matmul(
                t1_ps[e], lhsT=gh[f][:, e * P : (e + 1) * P],
                rhs=wslice(wkh_sb, f, 0, C), start=f == 0, stop=False,
            )
    for f in range(CH):
        for e in range(CH):
            nc.tensor.matmul(
                t1_ps[e], lhsT=gh[f][:, e * P : (e + 1) * P],
                rhs=wslice(wkl_sb, f, 0, C), start=False, stop=False,
            )
    for f in range(CH):
        for e in range(CH):
            nc.tensor.matmul(
                t1_ps[e], lhsT=gl[f][:, e * P : (e + 1) * P],
                rhs=wslice(wkh_sb, f, 0, C), start=False, stop=f == CH - 1,
            )
    t1h = [sb.tile([P, C], F16, name=f"t1h{i}", tag=f"t1h{i}") for i in range(CH)]
    t1l = [sb.tile([P, C], F16, name=f"t1l{i}", tag=f"t1l{i}") for i in range(CH)]
    for e in range(CH):
        nc.scalar.copy(t1h[e], t1_ps[e])
        nc.vector.tensor_sub(t1l[e], t1_ps[e], t1h[e])

    # ---- logits = Wq T1 + rank-1 bias terms (fp32, PSUM-accumulated) ----
    l_ps = [ps.tile([P, C], F32, name=f"lps{i}", tag=f"pb{i}") for i in range(CH)]
    for c in range(CH):
        for e in range(CH):
            nc.tensor.matmul(
                l_ps[c], lhsT=wslice(wqh_sb, e, c * P, (c + 1) * P),
                rhs=t1h[e], start=e == 0, stop=False,
            )
        for e in range(CH):
            nc.tensor.matmul(
                l_ps[c], lhsT=wslice(wqh_sb, e, c * P, (c + 1) * P),
                rhs=t1l[e], start=False, stop=False,
            )
        for e in range(CH):
            nc.tensor.matmul(
                l_ps[c], lhsT=wslice(wql_sb, e, c * P, (c + 1) * P),
                rhs=t1h[e], start=False, stop=False,
            )
        nc.tensor.matmul(
            l_ps[c], lhsT=lhs2[:, c * P : (c + 1) * P], rhs=rhs2,
            start=False, stop=True,
        )

    # ---- softmax over rows of L ----
    w16_sb = [sb.tile([P, C], F16, name=f"w16_{i}", tag=f"w16_{i}") for i in range(CH)]
    for c in range(CH):
        negmx = sb.tile([P, 1], F32, name=f"negmx{c}", tag=f"negmx{c}")
        nc.vector.reduce_max(negmx, l_ps[c], axis=AX, negate=True)
        e_sb = sb.tile([P, C], F32, name="esb", tag="esb", bufs=2)
        ssum = sb.tile([P, 1], F32, name=f"ssum{c}", tag=f"ssum{c}")
        nc.scalar.activation(e_sb, l_ps[c], EXP, bias=negmx, scale=1.0, accum_out=ssum)
        rcp = sb.tile([P, 1], F32, name=f"rcp{c}", tag=f"rcp{c}")
        nc.vector.reciprocal(rcp, ssum)
        nc.vector.tensor_scalar_mul(w16_sb[c], e_sb, rcp)

    # ---- transpose softmax weights (fp16, PE) ----
    wt_sb = [sb.tile([P, C], F16, name=f"wtsb{j}", tag=f"wtsb{j}") for j in range(CH)]
    for j in range(CH):
        wt_ps = ps.tile([P, C], F16, name=f"wtps{j}", tag=f"pb{j}")
        for i in range(CH):
            nc.tensor.transpose(
                wt_ps[:, i * P : (i + 1) * P],
                w16_sb[i][:, j * P : (j + 1) * P],
                id16_sb,
            )
        nc.vector.tensor_copy(wt_sb[j], wt_ps)

    # ---- out = w v + x (fp16 matmuls, residual from fp16 x) ----
    def out_tile(nt, fine=False):
        for c in range(CH):
            o_ps = ps.tile([P, 512], F32, name=f"ops{c}", tag=f"pb{c}")
            for d in range(CH):
                nc.tensor.matmul(
                    o_ps,
                    lhsT=wt_sb[d][:, c * P : (c + 1) * P],
                    rhs=v_sb[d][:, nt * 512 : (nt + 1) * 512],
                    start=d == 0,
                    stop=d == CH - 1,
                )
            o_sb = so.tile([P, 512], F32, name="osb", tag="osb", bufs=4)
            pieces = ((0, 256), (256, 512)) if (fine and c == CH - 1) else ((0, 512),)
            for lo, hi in pieces:
                nc.vector.tensor_add(
                    o_sb[:, lo:hi], o_ps[:, lo:hi],
                    x16_sb[c][:, nt * 512 + lo : nt * 512 + hi],
                )
                nc.sync.dma_start(
                    out[c * P : (c + 1) * P, nt * 512 + lo : nt * 512 + hi],
                    o_sb[:, lo:hi],
                )

    out_tile(0)
    v_conv(7, "pa")
    for nt in range(1, NT):
        out_tile(nt)

    for pool in (so, st, sb, ps):
        pool.release()


def _build_nc(repeat=1):
    nc = bacc.Bacc(
        "TRN2",
        target_bir_lowering=False,
        debug=False,
        num_devices=B,
        enable_asserts=False,
    )
    io = {}
    dt = nc.dram_tensor
    io["xt16"] = dt("xt16", (HW, C), F16, kind="ExternalInput").ap()
    io["x16"] = dt("x16", (C, HW), F16, kind="ExternalInput").ap()
    io["wqh"] = dt("wqh", (C, C), F16, kind="ExternalInput").ap()
    io["wql"] = dt("wql", (C, C), F16, kind="ExternalInput").ap()
    io["wkh"] = dt("wkh", (C, C), F16, kind="ExternalInput").ap()
    io["wkl"] = dt("wkl", (C, C), F16, kind="ExternalInput").ap()
    io["wvt"] = dt("wvt", (C, C), F16, kind="ExternalInput").ap()
    io["bqr"] = dt("bqr", (1, C), F32, kind="ExternalInput").ap()
    io["bkr"] = dt("bkr", (1, C), F32, kind="ExternalInput").ap()
    io["nbkr"] = dt("nbkr", (1, C), F32, kind="ExternalInput").ap()
    io["bvc"] = dt("bvc", (C, 1), F32, kind="ExternalInput").ap()
    io["id16"] = dt("id16", (P, P), F16, kind="ExternalInput").ap()
    io["out"] = dt("out", (C, HW), F32, kind="ExternalOutput").ap()
    with tile.TileContext(nc) as tc:
        for _ in range(repeat):
            _body(tc, nc, io)
    nc.compile()
    return nc


_NC_CACHE = None


def get_nc():
    global _NC_CACHE
    if _NC_CACHE is None:
        _NC_CACHE = _build_nc()
    return _NC_CACHE


def prep_in_maps(x, wq, bq, wk, bk, wv, bv):
    """Host-side input prep: reshape/transpose/dtype casts only."""
    x = np.asarray(x, dtype=np.float32)
    X = x.reshape(B, C, HW)
    XT = np.ascontiguousarray(X.transpose(0, 2, 1))
    xt16 = XT.astype(np.float16)
    x16 = X.astype(np.float16)
    wqt = np.ascontiguousarray(np.asarray(wq, np.float32).T)
    wkt = np.ascontiguousarray(np.asarray(wk, np.float32).T)
    wqh = wqt.astype(np.float16)
    wql = (wqt - wqh.astype(np.float32)).astype(np.float16)
    wkh = wkt.astype(np.float16)
    wkl = (wkt - wkh.astype(np.float32)).astype(np.float16)
    wvt = np.ascontiguousarray(np.asarray(wv, np.float32).T).astype(np.float16)
    bqr = np.asarray(bq, np.float32).reshape(1, C)
    bkr = np.asarray(bk, np.float32).reshape(1, C)
    nbkr = (float(HW) * np.asarray(bk, np.float32)).reshape(1, C)
    bvc = np.asarray(bv, np.float32).reshape(C, 1)
    id16 = np.eye(P, dtype=np.float16)
    in_maps = []
    for b in range(B):
        in_maps.append(
            {
                "xt16": xt16[b],
                "x16": np.ascontiguousarray(x16[b]),
                "wqh": wqh,
                "wql": wql,
                "wkh": wkh,
                "wkl": wkl,
                "wvt": wvt,
                "bqr": bqr,
                "bkr": bkr,
                "nbkr": nbkr,
                "bvc": bvc,
                "id16": id16,
            }
        )
    return in_maps


def kernel(x, wq, bq, wk, bk, wv, bv):
    nc = get_nc()
    in_maps = prep_in_maps(x, wq, bq, wk, bk, wv, bv)
    res = run_bass_kernel_spmd(nc, in_maps, core_ids=list(range(B)))
    out = np.stack([res.results[b]["out"] for b in range(B)])
    return out.reshape(B, C, 64, 64).astype(np.float32)



# revision 3
# speedup vs baseline: 1.3224x; 1.3224x over previous
"""Channel-attention (nn_ChannelAttentionModule) Trainium2 kernel.

Math (per batch b):
    X = x[b]  [C, N]  with C=512, N=64*64=4096
    q = Wq X + bq ; k = Wk X + bk ; v = Wv X + bv
    L = q k^T                       [C, C]
    A = softmax(L, -1)
    out = A v + X                   [C, N]

Restructure 1 (logits): L = Wq G Wk^T + u1 bk^T + bq (Wk S + N bk)^T with
G = X X^T (Gram, block-upper-triangle on the PE + mirrored via transposes),
S = X 1, u1 = Wq S.  G is one fp16 pass (fp32 PSUM); G Wk^T and Wq T1 are
single-pass fp32r matmuls (1 cyc/row at >=256-wide, ~2^-12 rounding).

Restructure 2 (v path): out = (A Wv + I) X + (A bv) 1^T, which removes the
whole Wv X conv (C^2 N MACs) and replaces it with the C^3 product A Wv; the
residual is folded into the diagonal and A bv rides the PSUM evacuation bias.

Sharding: pure data-parallel, one batch per NeuronCore (B=8, 8 cores).
"""

import numpy as np

import concourse.mybir as mybir
import concourse.tile as tile
from concourse import bacc
from concourse.bass_utils import run_bass_kernel_spmd

F32 = mybir.dt.float32
F32R = mybir.dt.float32r
F16 = mybir.dt.float16
AX = mybir.AxisListType.X
EXP = mybir.ActivationFunctionType.Exp

B = 8
C = 512
HW = 64 * 64
P = 128
CH = C // P  # 4 channel chunks
NG = 8  # xtr granules (4 spatial tiles of 512 each)
# upper-triangle start per G row chunk (tight block triangle)
USTART = [0, 128, 256, 384]


def _body(tc, nc, io):
    xt16, x16 = io["xt16"], io["x16"]
    wqt, wkt, wv16 = io["wqt"], io["wkt"], io["wv16"]
    bqr, bkr, nbkr, bv16 = io["bqr"], io["bkr"], io["nbkr"], io["bv16"]
    id16, id32, out = io["id16"], io["id32"], io["out16"]

    ps = tc.alloc_tile_pool(name="ps", bufs=1, space="PSUM")
    sb = tc.alloc_tile_pool(name="sb", bufs=1)
    so = tc.alloc_tile_pool(name="so", bufs=2)

    # ---- persistent SBUF tiles ----
    x16_sb = [sb.tile([P, HW], F16, name=f"x16_{i}", tag=f"x16_{i}") for i in range(CH)]
    ar_sb = [sb.tile([P, 4 * C], F16, name=f"ar{g}", tag=f"ar{g}") for g in range(NG)]
    wqt_sb = sb.tile([P, CH * C], F32R, name="wqt_sb", tag="wqt_sb")
    wkt_sb = sb.tile([P, CH * C], F32R, name="wkt_sb", tag="wkt_sb")
    wv_sb = sb.tile([P, CH * C], F16, name="wv_sb", tag="wv_sb")
    id16_sb = sb.tile([P, P], F16, name="id16sb", tag="id16sb")
    id32_sb = sb.tile([P, P], F32R, name="id32sb", tag="id32sb")
    bv_sb = sb.tile([P, CH], F16, name="bv_sb", tag="bv_sb")
    nbkr_sb = sb.tile([1, C], F32, name="nbkrsb", tag="nbkrsb")
    lhs2 = sb.tile([2, C], F32R, name="lhs2", tag="lhs2")
    rhs2 = sb.tile([2, C], F32R, name="rhs2", tag="rhs2")

    xtr3 = xt16.rearrange("(g t p) c -> g p t c", p=P, t=4)

    def xtr_load(g2):
        nc.sync.dma_start(ar_sb[g2].rearrange("p (t c) -> p t c", t=4), xtr3[g2])

    g_ps = [ps.tile([P, C], F32, name=f"gps{i}", tag=f"pa{i}") for i in range(CH)]

    def g_pass(g2):
        ar4 = ar_sb[g2]
        for t in range(4):
            n = g2 * 4 + t
            first, last = n == 0, n == 4 * NG - 1
            for c in range(CH):
                u = USTART[c]
                nc.tensor.matmul(
                    g_ps[c][:, u:],
                    lhsT=ar4[:, t * C + c * P : t * C + (c + 1) * P],
                    rhs=ar4[:, t * C + u : (t + 1) * C],
                    start=first,
                    stop=last,
                )

    def x16_load(nt2):
        for c in range(CH):
            nc.gpsimd.dma_start(
                x16_sb[c][:, nt2 * 1024 : (nt2 + 1) * 1024],
                x16[c * P : (c + 1) * P, nt2 * 1024 : (nt2 + 1) * 1024],
            )

    s4 = [sb.tile([P, 4], F32, name=f"s4_{i}", tag=f"s4_{i}") for i in range(CH)]

    def s_reduce(nt2):
        for c in range(CH):
            nc.vector.reduce_sum(
                s4[c][:, nt2 : nt2 + 1],
                x16_sb[c][:, nt2 * 1024 : (nt2 + 1) * 1024],
                axis=AX,
            )

    # ---- front: xtr stream (sync q) + consts/weights/x16 (gpsimd q) + G ----
    nc.sync.dma_start(
        ar_sb[0][:, 0 : 2 * C].rearrange("p (t c) -> p t c", t=2), xtr3[0][:, 0:2]
    )
    nc.sync.dma_start(
        ar_sb[0][:, 2 * C :].rearrange("p (t c) -> p t c", t=2), xtr3[0][:, 2:4]
    )
    xtr_load(1)
    nc.gpsimd.dma_start(id16_sb, id16)
    nc.gpsimd.dma_start(id32_sb, id32)
    nc.gpsimd.dma_start(bv_sb, bv16)
    nc.gpsimd.dma_start(lhs2[0:1, :], bqr)
    nc.gpsimd.dma_start(rhs2[1:2, :], bkr)
    nc.gpsimd.dma_start(nbkr_sb, nbkr)
    nc.gpsimd.dma_start(
        wkt_sb.rearrange("p (e c) -> p e c", e=CH),
        wkt.rearrange("(e p) c -> p e c", p=P),
    )
    g_pass(0)
    xtr_load(2)
    nc.gpsimd.dma_start(
        wqt_sb.rearrange("p (e c) -> p e c", e=CH),
        wqt.rearrange("(e p) c -> p e c", p=P),
    )
    g_pass(1)
    xtr_load(3)
    nc.gpsimd.dma_start(
        wv_sb.rearrange("p (e c) -> p e c", e=CH),
        wv16.rearrange("(e p) c -> p e c", p=P),
    )
    g_pass(2)
    xtr_load(4)
    x16_load(0)
    s_reduce(0)
    g_pass(3)
    xtr_load(5)
    x16_load(1)
    s_reduce(1)
    g_pass(4)
    xtr_load(6)
    x16_load(2)
    s_reduce(2)
    g_pass(5)
    xtr_load(7)
    x16_load(3)
    s_reduce(3)
    g_pass(6)
    g_pass(7)

    # ---- S column + f32r copy (feeds u1/u2) ----
    s_col = [sb.tile([P, 1], F32, name=f"s{i}", tag=f"s{i}") for i in range(CH)]
    s32r = [sb.tile([P, 1], F32R, name=f"sr{i}", tag=f"sr{i}") for i in range(CH)]
    for i in range(CH):
        nc.vector.reduce_sum(s_col[i], s4[i], axis=AX)
        nc.scalar.copy(s32r[i], s_col[i])

    # ---- stage G -> f32r SBUF, mirror lower blocks via f32r PE transposes ----
    g_sb = [sb.tile([P, C], F32R, name=f"gsb{i}", tag=f"gsb{i}") for i in range(CH)]
    for c in range(CH):
        u = USTART[c]
        nc.scalar.copy(g_sb[c][:, u:], g_ps[c][:, u:])
    mi = 0
    for c in range(1, CH):
        for d in range(c):
            tp = ps.tile([P, P], F32, name=f"mtp{c}{d}", tag=f"pb{mi % 2}")
            nc.tensor.transpose(
                tp.bitcast(F32R), g_sb[d][:, c * P : (c + 1) * P], id32_sb
            )
            nc.scalar.copy(g_sb[c][:, d * P : (d + 1) * P], tp)
            mi += 1

    # ---- T1 = G Wk^T, single-pass fp32r ----
    t1_ps = [ps.tile([P, C], F32, name=f"t1ps{i}", tag=f"pb{i}") for i in range(CH)]
    for f in range(CH):
        for e in range(CH):
            nc.tensor.matmul(
                t1_ps[e],
                lhsT=g_sb[f][:, e * P : (e + 1) * P],
                rhs=wkt_sb[:, f * C : (f + 1) * C],
                start=f == 0,
                stop=f == CH - 1,
            )

    # ---- u1 = Wq S, u2 = Wk S (fp32r) + rank-1 operand rows ----
    u1_ps = ps.tile([1, C], F32, name="u1ps", tag="pa0")
    u2_ps = ps.tile([1, C], F32, name="u2ps", tag="pa1")
    for e in range(CH):
        nc.tensor.matmul(
            u1_ps, lhsT=s32r[e], rhs=wqt_sb[:, e * C : (e + 1) * C],
            start=e == 0, stop=e == CH - 1,
        )
    for e in range(CH):
        nc.tensor.matmul(
            u2_ps, lhsT=s32r[e], rhs=wkt_sb[:, e * C : (e + 1) * C],
            start=e == 0, stop=e == CH - 1,
        )
    rhs2f = sb.tile([1, C], F32, name="rhs2f", tag="rhs2f")
    nc.vector.tensor_add(rhs2f, u2_ps, nbkr_sb)
    nc.scalar.copy(rhs2[0:1, :], rhs2f)
    u1r = sb.tile([1, C], F32R, name="u1r", tag="u1r")
    nc.scalar.copy(u1r, u1_ps)
    nc.sync.dma_start(lhs2[1:2, :], u1r)

    # ---- stage T1 -> f32r SBUF ----
    t1_sb = [sb.tile([P, C], F32R, name=f"t1sb{i}", tag=f"t1sb{i}") for i in range(CH)]
    for e in range(CH):
        nc.scalar.copy(t1_sb[e], t1_ps[e])

    # ---- logits = Wq T1 + rank-1 (fp32r, PSUM-accumulated) ----
    l_ps = [ps.tile([P, C], F32, name=f"lps{i}", tag=f"pa{i}") for i in range(CH)]
    for c in range(CH):
        for e in range(CH):
            nc.tensor.matmul(
                l_ps[c],
                lhsT=wqt_sb[:, e * C + c * P : e * C + (c + 1) * P],
                rhs=t1_sb[e],
                start=e == 0,
                stop=False,
            )
        nc.tensor.matmul(
            l_ps[c], lhsT=lhs2[:, c * P : (c + 1) * P], rhs=rhs2,
            start=False, stop=True,
        )

    # ---- softmax over rows of L -> fp16 weights ----
    w16_sb = [sb.tile([P, C], F16, name=f"w16_{i}", tag=f"w16_{i}") for i in range(CH)]
    for c in range(CH):
        negmx = sb.tile([P, 1], F32, name=f"negmx{c}", tag=f"negmx{c}")
        nc.vector.reduce_max(negmx, l_ps[c], axis=AX, negate=True)
        e_sb = sb.tile([P, C], F32, name="esb", tag="esb", bufs=2)
        ssum = sb.tile([P, 1], F32, name=f"ssum{c}", tag=f"ssum{c}")
        nc.scalar.activation(e_sb, l_ps[c], EXP, bias=negmx, scale=1.0, accum_out=ssum)
        rcp = sb.tile([P, 1], F32, name=f"rcp{c}", tag=f"rcp{c}")
        nc.vector.reciprocal(rcp, ssum)
        nc.vector.tensor_scalar_mul(w16_sb[c], e_sb, rcp)

    # ---- transpose softmax weights (fp16, PE): wt_sb[j] = A^T chunk j ----
    wt_sb = [sb.tile([P, C], F16, name=f"wtsb{j}", tag=f"wtsb{j}") for j in range(CH)]
    for j in range(CH):
        wt_ps = ps.tile([P, C], F16, name=f"wtps{j}", tag=f"pb{j}")
        for i in range(CH):
            nc.tensor.transpose(
                wt_ps[:, i * P : (i + 1) * P],
                w16_sb[i][:, j * P : (j + 1) * P],
                id16_sb,
            )
        nc.vector.tensor_copy(wt_sb[j], wt_ps)

    # ---- MT = (A Wv)^T + I fold (fp16); r = A bv ----
    mt_sb = [sb.tile([P, C], F16, name=f"mtsb{i}", tag=f"mtsb{i}") for i in range(CH)]
    for cc in range(CH):
        mt_ps = ps.tile([P, C], F32, name=f"mtps{cc}", tag=f"pa{cc}")
        for oc in range(CH):
            nc.tensor.matmul(
                mt_ps,
                lhsT=wv_sb[:, oc * C + cc * P : oc * C + (cc + 1) * P],
                rhs=wt_sb[oc],
                start=oc == 0,
                stop=oc == CH - 1,
            )
        lo, hi = cc * P, (cc + 1) * P
        if cc > 0:
            nc.scalar.copy(mt_sb[cc][:, :lo], mt_ps[:, :lo])
        nc.vector.tensor_add(mt_sb[cc][:, lo:hi], mt_ps[:, lo:hi], id16_sb)
        if cc < CH - 1:
            nc.scalar.copy(mt_sb[cc][:, hi:], mt_ps[:, hi:])

    r_col = [sb.tile([P, 1], F32, name=f"rcol{i}", tag=f"rcol{i}") for i in range(CH)]
    for ic in range(CH):
        r_ps = ps.tile([P, 1], F32, name=f"rps{ic}", tag=f"pb{ic}")
        for oc in range(CH):
            nc.tensor.matmul(
                r_ps,
                lhsT=wt_sb[oc][:, ic * P : (ic + 1) * P],
                rhs=bv_sb[:, oc : oc + 1],
                start=oc == 0,
                stop=oc == CH - 1,
            )
        nc.vector.tensor_copy(r_col[ic], r_ps)

    # ---- out = MT'^T X + r (fp16 matmuls; evac alternates ACT/DVE) ----
    for nt in range(NG):
        bank = "pa" if nt % 2 == 0 else "pb"
        for ic in range(CH):
            o_ps = ps.tile([P, 512], F32, name=f"ops{ic}", tag=f"{bank}{ic}")
            for cc in range(CH):
                nc.tensor.matmul(
                    o_ps,
                    lhsT=mt_sb[cc][:, ic * P : (ic + 1) * P],
                    rhs=x16_sb[cc][:, nt * 512 : (nt + 1) * 512],
                    start=cc == 0,
                    stop=cc == CH - 1,
                )
            o_sb = so.tile([P, 512], F16, name="osb", tag="osb", bufs=4)
            if (nt + ic) % 2 == 0:
                nc.scalar.add(o_sb, o_ps, r_col[ic])
            else:
                nc.vector.tensor_scalar_add(o_sb, o_ps, r_col[ic])
            nc.sync.dma_start(
                out[ic * P : (ic + 1) * P, nt * 512 : (nt + 1) * 512], o_sb
            )

    for pool in (so, sb, ps):
        pool.release()


def _build_nc(repeat=1):
    nc = bacc.Bacc(
        "TRN2",
        target_bir_lowering=False,
        debug=False,
        num_devices=B,
        enable_asserts=False,
    )
    io = {}
    dt = nc.dram_tensor
    io["xt16"] = dt("xt16", (HW, C), F16, kind="ExternalInput").ap()
    io["x16"] = dt("x16", (C, HW), F16, kind="ExternalInput").ap()
    io["wqt"] = dt("wqt", (C, C), F32R, kind="ExternalInput").ap()
    io["wkt"] = dt("wkt", (C, C), F32R, kind="ExternalInput").ap()
    io["wv16"] = dt("wv16", (C, C), F16, kind="ExternalInput").ap()
    io["bqr"] = dt("bqr", (1, C), F32R, kind="ExternalInput").ap()
    io["bkr"] = dt("bkr", (1, C), F32R, kind="ExternalInput").ap()
    io["nbkr"] = dt("nbkr", (1, C), F32, kind="ExternalInput").ap()
    io["bv16"] = dt("bv16", (P, CH), F16, kind="ExternalInput").ap()
    io["id16"] = dt("id16", (P, P), F16, kind="ExternalInput").ap()
    io["id32"] = dt("id32", (P, P), F32R, kind="ExternalInput").ap()
    io["out16"] = dt("out16", (C, HW), F16, kind="ExternalOutput").ap()
    with tile.TileContext(nc) as tc:
        for _ in range(repeat):
            _body(tc, nc, io)
    nc.compile()
    return nc


_NC_CACHE = None


def get_nc():
    global _NC_CACHE
    if _NC_CACHE is None:
        _NC_CACHE = _build_nc()
    return _NC_CACHE


def prep_in_maps(x, wq, bq, wk, bk, wv, bv):
    """Host-side input prep: reshape/transpose/dtype casts only."""
    x = np.asarray(x, dtype=np.float32)
    X = x.reshape(B, C, HW)
    XT = np.ascontiguousarray(X.transpose(0, 2, 1))
    xt16 = XT.astype(np.float16)
    x16 = X.astype(np.float16)
    wqt = np.ascontiguousarray(np.asarray(wq, np.float32).T)
    wkt = np.ascontiguousarray(np.asarray(wk, np.float32).T)
    wv16 = np.asarray(wv, np.float32).astype(np.float16)
    bqr = np.asarray(bq, np.float32).reshape(1, C)
    bkr = np.asarray(bk, np.float32).reshape(1, C)
    nbkr = (float(HW) * np.asarray(bk, np.float32)).reshape(1, C)
    bv16 = np.ascontiguousarray(
        np.asarray(bv, np.float32).reshape(CH, P).T
    ).astype(np.float16)
    id16 = np.eye(P, dtype=np.float16)
    id32 = np.eye(P, dtype=np.float32)
    in_maps = []
    for b in range(B):
        in_maps.append(
            {
                "xt16": xt16[b],
                "x16": np.ascontiguousarray(x16[b]),
                "wqt": wqt,
                "wkt": wkt,
                "wv16": wv16,
                "bqr": bqr,
                "bkr": bkr,
                "nbkr": nbkr,
                "bv16": bv16,
                "id16": id16,
                "id32": id32,
            }
        )
    return in_maps


def kernel(x, wq, bq, wk, bk, wv, bv):
    nc = get_nc()
    in_maps = prep_in_maps(x, wq, bq, wk, bk, wv, bv)
    res = run_bass_kernel_spmd(nc, in_maps, core_ids=list(range(B)))
    out = np.stack([res.results[b]["out16"] for b in range(B)])
    return out.reshape(B, C, 64, 64).astype(np.float32)
